# revision 39
# baseline (speedup 1.0000x reference)
"""MultiHeadGraphAttention TRN2 kernel.

Data-parallel over (batch, query-half): core c handles batch c//2, query rows
(c%2)*1024 .. +1024.  Attention rows are independent, so there are no
collectives.  All matmuls run in bf16 (fp32 PSUM accumulation); softmax and
LayerNorm run in fp32.

Engine budget per core (the design target is max-engine, not sum):
  ScalarE: 128 exp ops of [128,1024] (the softmax) ~= 142us  <- pacer
  VectorE: mask multiplies (bf16 2x mode), PSUM evacuations, LayerNorm
  TensorE: projections + scores + AV + out-proj, kept dense (HAM-warm)

Key structural points:
  - scores are computed TRANSPOSED (S^T[m,n]) so softmax needs no on-chip
    transposes; the denominator comes free from a ones-column appended to V.
  - ScalarE uses ONLY {Exp, Ln} which live in one activation-table set
    (natural_log_exp_and_others): rsqrt(v) = exp(-0.5*ln(v)) and
    1/den = exp(-ln(den)).  No table switches anywhere.
  - per-head denominators are collected at partitions {32a, 32a+1} of a
    [98, ...] tile, reciprocated in 2 batched ScalarE ops per chunk, and
    broadcast to 128 partitions with a tiny contraction-2 PE matmul.
  - DMAs are emitted in consumption order (weights first); attention on
    head 0 starts as soon as Q(t=0)/K(t=0) land, with the remaining
    projections interleaved as fillers so the PE never idles.
"""

import os
import sys

import numpy as np

try:
    import concourse  # noqa: F401
except ImportError:  # harness runs from a bare dir; the repo is a fixed path
    sys.path.insert(0, "/opt/trn_rl_repo")

import ml_dtypes

B, N, M, D, H, HD = 4, 2048, 2048, 512, 8, 64
NS = 1024          # query rows per core
NCORES = 8
LN_EPS = 1e-5
BF16 = ml_dtypes.bfloat16

_CACHE = {}


def _build(affine: bool):
    import concourse.bass as bass  # noqa: F401
    import concourse.tile as tile
    from concourse import bacc, mybir

    f32 = mybir.dt.float32
    bf16 = mybir.dt.bfloat16
    Exp = mybir.ActivationFunctionType.Exp
    Ln = mybir.ActivationFunctionType.Ln

    # The act-table insertion pass picks the FIRST set containing each
    # function, which alternates exp_and_others / natural_log and inserts a
    # ~1.3us ACT_TABLE_LOAD per switch (23 of them).  Steer it to the one
    # set that contains BOTH exp and ln by stripping those functions from
    # the other sets in the cached table dict.  Keys/order are unchanged,
    # so act_func_set_id indices stay valid; the real on-device tables are
    # untouched (worst case is a redundant load, never wrong tables).
    from concourse.hw_specs import get_activation_tables
    sub = mybir.AluOpType.subtract
    mult = mybir.AluOpType.mult

    nc = bacc.Bacc(None, target_bir_lowering=False, debug=False)

    tables = get_activation_tables(nc.m.arch)
    combined = "natural_log_exp_and_others"
    if combined in tables:
        for name, funcs in tables.items():
            if name != combined:
                funcs.discard(Exp)
                funcs.discard(Ln)

    xqT_d = nc.dram_tensor("xqT", [D, NS], bf16, kind="ExternalInput")
    xkT_d = nc.dram_tensor("xkT", [D, M], bf16, kind="ExternalInput")
    xvT_d = nc.dram_tensor("xvT", [D, M], bf16, kind="ExternalInput")
    maskT_d = nc.dram_tensor("maskT", [M, NS], bf16, kind="ExternalInput")
    qres_d = nc.dram_tensor("qres", [NS, D], bf16, kind="ExternalInput")
    wqT_d = nc.dram_tensor("wqT", [D, D], bf16, kind="ExternalInput")
    wkT_d = nc.dram_tensor("wkT", [D, D], bf16, kind="ExternalInput")
    wvT_d = nc.dram_tensor("wvT", [D, D], bf16, kind="ExternalInput")
    woT_d = nc.dram_tensor("woT", [D, D], bf16, kind="ExternalInput")
    if affine:
        gamma_d = nc.dram_tensor("gamma", [1, D], f32, kind="ExternalInput")
        beta_d = nc.dram_tensor("beta", [1, D], f32, kind="ExternalInput")
    out_d = nc.dram_tensor("out", [NS, D], f32, kind="ExternalOutput")

    KC = D // 128      # 4 contraction chunks of 128
    NCH = NS // 512    # 2 query-column chunks
    MT = M // 128      # 16 key-position tiles
    MCH = M // 512     # 4 key chunks of 512

    xqT_r = xqT_d[:].rearrange("(c p) n -> p c n", p=128)
    xkT_r = xkT_d[:].rearrange("(c p) n -> p c n", p=128)
    xvT_r = xvT_d[:].rearrange("(c p) n -> p c n", p=128)
    maskT_r = maskT_d[:].rearrange("(j p) n -> p j n", p=128)
    qres_r = qres_d[:].rearrange("(t p) d -> p t d", p=128)
    out_r = out_d[:].rearrange("(t p) d -> p t d", p=128)

    with tile.TileContext(nc) as tc:
        with (
            tc.tile_pool(name="big", bufs=1) as big,
            tc.tile_pool(name="wpool", bufs=1) as wpool,
            tc.tile_pool(name="ppool", bufs=5) as ppool,
            tc.tile_pool(name="ypool", bufs=4) as ypool,
            tc.tile_pool(name="small", bufs=4) as small,
            tc.tile_pool(name="ps_s", bufs=2, space="PSUM") as ps_s,
            tc.tile_pool(name="ps_o", bufs=2, space="PSUM") as ps_o,
            tc.tile_pool(name="ps_mm", bufs=2, space="PSUM") as ps_mm,
        ):
            # ---- resident SBUF tensors -----------------------------------
            xqT = big.tile([128, KC, NS], bf16, tag="xqT")
            xkT = big.tile([128, KC, M], bf16, tag="xkT")
            xvT = big.tile([128, KC, M], bf16, tag="xvT")
            maskT = big.tile([128, MT, NS], bf16, tag="maskT")
            qT = big.tile([128, KC, NS], bf16, tag="qT")
            kT = big.tile([128, KC, M], bf16, tag="kT")
            vS = big.tile([128, MT, H * (HD + 1)], bf16, tag="vS")
            oT_u = big.tile([128, KC, NS], bf16, tag="oT_u")
            # denominators: head pair (2a, 2a+1) lives on partition 32a
            # (parity = second dim) so every engine AP base is 32-aligned
            den_q = big.tile([98, 2, NCH, 512], f32, tag="den_q")
            recip_q = big.tile([98, 2, NCH, 512], bf16, tag="recip_q")
            wq = wpool.tile([128, KC, D], bf16, tag="wq")
            wk = wpool.tile([128, KC, D], bf16, tag="wk")
            wv = wpool.tile([128, KC, D], bf16, tag="wv")
            wo = wpool.tile([128, KC, D], bf16, tag="wo")
            sel_t = wpool.tile([98, 2, 128], bf16, tag="sel_t")
            eps_t = wpool.tile([128, 1], f32, tag="eps")
            warm = wpool.tile([1, 2], f32, tag="warm")
            warm_mm = wpool.tile([128, 512], bf16, tag="warm_mm")
            qres_sb = big.tile([128, 8, 512], bf16, tag="qres_sb")
            if affine:
                gamma_1 = wpool.tile([1, D], f32, tag="gamma_1")
                beta_1 = wpool.tile([1, D], f32, tag="beta_1")
                gamma_b = wpool.tile([128, D], f32, tag="gamma_b")
                beta_b = wpool.tile([128, D], f32, tag="beta_b")

            # constants + ACT table warm-up (loads the ln/exp table set at
            # t~0, overlapping the input-DMA ramp)
            nc.vector.memset(eps_t, LN_EPS)
            nc.scalar.activation(warm[0:1, 0:1], eps_t[0:1, 0:1], Exp)
            nc.scalar.activation(warm[0:1, 1:2], eps_t[0:1, 0:1], Ln)
            # PE warm-up: ~4.5us of dummy matmuls before the first DMAs land
            # so HAM reaches K=8/8 by the time real projections start
            nc.vector.memset(warm_mm, 0.0)
            for _ in range(12):
                psw = ps_mm.tile([128, 512], f32, tag="mm")
                for _ in range(2):
                    nc.tensor.matmul(
                        psw, lhsT=warm_mm[:, 0:128], rhs=warm_mm,
                        start=True, stop=True,
                    )
                # tiny reader so DCE keeps the warm-up matmuls
                nc.scalar.copy(warm[0:1, 0:1], psw[0:1, 0:1])
            nc.vector.memset(sel_t, 0.0)
            for a in range(4):
                nc.vector.memset(sel_t[32 * a : 32 * a + 1, 0, 0:64], 1.0)
                nc.vector.memset(sel_t[32 * a : 32 * a + 1, 1, 64:128], 1.0)
            # ones column per head in augmented V -> softmax denominator
            # lands as row 64 of the AV matmul output
            nc.vector.memset(
                vS[:].rearrange("p j (h x) -> p j h x", x=HD + 1)[:, :, :, HD : HD + 1],
                1.0,
            )

            # ---- input DMAs in consumption order -------------------------
            def dma_mask(j):
                nc.sync.dma_start(out=maskT[:, j, :], in_=maskT_r[:, j, :])

            wq_r = wqT_d[:].rearrange("(c p) o -> p c o", p=128)
            wk_r = wkT_d[:].rearrange("(c p) o -> p c o", p=128)
            nc.sync.dma_start(out=wq[:, :, 0:128], in_=wq_r[:, :, 0:128])
            nc.sync.dma_start(out=xqT[:, :, 0:512], in_=xqT_r[:, :, 0:512])
            nc.sync.dma_start(out=wk[:, :, 0:128], in_=wk_r[:, :, 0:128])
            nc.sync.dma_start(out=xkT[:, :, 0:512], in_=xkT_r[:, :, 0:512])
            nc.sync.dma_start(out=wq[:, :, 128:512], in_=wq_r[:, :, 128:512])
            nc.sync.dma_start(out=wk[:, :, 128:512], in_=wk_r[:, :, 128:512])
            nc.sync.dma_start(out=wv, in_=wvT_d[:].rearrange("(c p) o -> p c o", p=128))
            nc.sync.dma_start(out=xvT[:, :, 0:512], in_=xvT_r[:, :, 0:512])
            nc.sync.dma_start(out=xkT[:, :, 512:1024], in_=xkT_r[:, :, 512:1024])
            dma_mask(0)
            dma_mask(1)
            nc.sync.dma_start(out=xvT[:, :, 512:1024], in_=xvT_r[:, :, 512:1024])
            nc.sync.dma_start(out=xkT[:, :, 1024:1536], in_=xkT_r[:, :, 1024:1536])
            dma_mask(2)
            dma_mask(3)
            nc.sync.dma_start(out=xkT[:, :, 1536:2048], in_=xkT_r[:, :, 1536:2048])
            nc.sync.dma_start(out=xvT[:, :, 1024:1536], in_=xvT_r[:, :, 1024:1536])
            dma_mask(4)
            dma_mask(5)
            nc.sync.dma_start(out=xvT[:, :, 1536:2048], in_=xvT_r[:, :, 1536:2048])
            for j in range(6, 10):
                dma_mask(j)
            nc.sync.dma_start(out=wo, in_=woT_d[:].rearrange("(c p) o -> p c o", p=128))
            nc.sync.dma_start(out=qres_sb, in_=qres_r)
            for j in range(10, MT):
                dma_mask(j)
            nc.sync.dma_start(out=xqT[:, :, 512:1024], in_=xqT_r[:, :, 512:1024])
            if affine:
                nc.sync.dma_start(out=gamma_1, in_=gamma_d[:])
                nc.sync.dma_start(out=beta_1, in_=beta_d[:])
                nc.gpsimd.partition_broadcast(gamma_b, gamma_1, channels=128)
                nc.gpsimd.partition_broadcast(beta_b, beta_1, channels=128)

            # ---- projection emitters (PSUM from ps_mm, evac on VectorE) --
            def q_proj(t, ncc):
                ps = ps_mm.tile([128, 512], f32, tag="mm")
                for kc in range(KC):
                    nc.tensor.matmul(
                        ps,
                        lhsT=wq[:, kc, t * 128 : (t + 1) * 128],
                        rhs=xqT[:, kc, ncc * 512 : (ncc + 1) * 512],
                        start=(kc == 0),
                        stop=(kc == KC - 1),
                    )
                nc.scalar.copy(qT[:, t, ncc * 512 : (ncc + 1) * 512], ps)

            def k_proj(t, mc):
                ps = ps_mm.tile([128, 512], f32, tag="mm")
                for kc in range(KC):
                    nc.tensor.matmul(
                        ps,
                        lhsT=wk[:, kc, t * 128 : (t + 1) * 128],
                        rhs=xkT[:, kc, mc * 512 : (mc + 1) * 512],
                        start=(kc == 0),
                        stop=(kc == KC - 1),
                    )
                nc.scalar.copy(kT[:, t, mc * 512 : (mc + 1) * 512], ps)

            def v_proj(j):
                ps = ps_mm.tile([128, 512], f32, tag="mm")
                for kc in range(KC):
                    nc.tensor.matmul(
                        ps,
                        lhsT=xvT[:, kc, j * 128 : (j + 1) * 128],
                        rhs=wv[:, kc, :],
                        start=(kc == 0),
                        stop=(kc == KC - 1),
                    )
                nc.vector.tensor_copy(
                    vS[:, j, :].rearrange("p (h x) -> p h x", x=HD + 1)[:, :, 0:HD],
                    ps[:].rearrange("p (h x) -> p h x", x=HD),
                )

            # ---- attention: one (head-pair, n-chunk) ---------------------
            # Heads 2p (PE rows 0-63) and 2p+1 (rows 64-127) are processed
            # together: their two K=64 score matmuls land in adjacent PE
            # queue slots on disjoint row groups, so the array runs them
            # CONCURRENTLY (row tiling) -- halving score PE time.
            def chunk(ncc, fillers):
                """All 4 head-pairs of one n-chunk, as a flat (pair, j) loop
                with a cross-pair AV pipeline: the next pair's score matmuls
                issue before the previous pair's last AVs + evacuation, so
                the PE stream never head-blocks at a pair boundary.
                fillers: {(p, j): [thunks]} emitted between scores and exp."""
                nsl = slice(ncc * 512, (ncc + 1) * 512)
                pend_av = []
                pend_evac = []

                def flush_av():
                    for pt_, j_, pa_, pb_, p_ in pend_av:
                        base = (j_ % 2) * 1024
                        for po_t, h, off in ((pa_, 2 * p_, 0), (pb_, 2 * p_ + 1, 512)):
                            nc.tensor.matmul(
                                po_t,
                                lhsT=vS[:, j_, h * (HD + 1) : (h + 1) * (HD + 1)],
                                rhs=pt_[:, base + off : base + off + 512],
                                start=(j_ == 0),
                                stop=(j_ == MT - 1),
                            )
                    pend_av.clear()

                def flush_evac():
                    for pa_, pb_, p_ in pend_evac:
                        nc.vector.tensor_copy(
                            den_q[32 * p_ : 32 * p_ + 1, 0, ncc, :], pa_[64:65, :]
                        )
                        nc.vector.tensor_copy(
                            den_q[32 * p_ : 32 * p_ + 1, 1, ncc, :], pb_[64:65, :]
                        )
                        nc.vector.tensor_copy(oT_u[0:64, p_, nsl], pa_[0:64, :])
                        nc.vector.tensor_copy(oT_u[64:128, p_, nsl], pb_[0:64, :])
                    pend_evac.clear()

                for p in range(4):
                    po_a = ps_o.tile([HD + 1, 512], f32, tag="po")
                    po_b = ps_o.tile([HD + 1, 512], f32, tag="po")
                    pt = None
                    for j in range(MT):
                        if j % 2 == 0:
                            pt = ppool.tile([128, 4 * 512], bf16, tag="pt")
                        psP = ps_s.tile([128, 1024], f32, tag="s")
                        nc.tensor.matmul(
                            psP[:, 0:512],
                            lhsT=kT[0:64, p, j * 128 : (j + 1) * 128],
                            rhs=qT[0:64, p, nsl],
                            start=True,
                            stop=True,
                        )
                        nc.tensor.matmul(
                            psP[:, 512:1024],
                            lhsT=kT[64:128, p, j * 128 : (j + 1) * 128],
                            rhs=qT[64:128, p, nsl],
                            start=True,
                            stop=True,
                        )
                        if j % 2 == 0:
                            # flush the previous j-pair's AVs one step earlier
                            # so pt tiles recycle sooner
                            flush_av()
                            flush_evac()
                        for f in fillers.get((p, j), ()):
                            f()
                        nc.scalar.activation(
                            pt[:, (j % 2) * 1024 : (j % 2 + 1) * 1024], psP, Exp
                        )
                        if j % 2 == 1:
                            ptv = pt[:].rearrange("p (a b n) -> p a b n", a=2, b=2)
                            nc.vector.tensor_mul(
                                ptv,
                                ptv,
                                maskT[:, j - 1 : j + 1, None, nsl].broadcast_to(
                                    [128, 2, 2, 512]
                                ),
                            )
                            flush_av()
                            flush_evac()
                            pend_av.append((pt, j - 1, po_a, po_b, p))
                            pend_av.append((pt, j, po_a, po_b, p))
                    pend_evac.append((po_a, po_b, p))
                flush_av()
                flush_evac()

            # ---- denominator reciprocal + normalize ----------------------
            def recip(ncc, p0=0, p1=98):
                # in-place ln, then recip = exp(-ln(den)); one ACT table set
                nc.scalar.activation(
                    den_q[p0:p1, :, ncc, :], den_q[p0:p1, :, ncc, :], Ln
                )
                nc.scalar.activation(
                    recip_q[p0:p1, :, ncc, :], den_q[p0:p1, :, ncc, :], Exp, scale=-1.0
                )

            def norm(a, ncc):
                nsl = slice(ncc * 512, (ncc + 1) * 512)
                sa = 32 * a
                ps = ps_mm.tile([128, 512], f32, tag="mm")
                # broadcast the pair's reciprocals to 128 partitions with two
                # chained K=1 matmuls: rows 0-63 <- head 2a, rows 64-127
                # (accumulated) <- head 2a+1
                nc.tensor.matmul(
                    ps,
                    lhsT=sel_t[sa : sa + 1, 0, :],
                    rhs=recip_q[sa : sa + 1, 0, ncc, :],
                    start=True,
                    stop=False,
                    tile_position=(sa, 0),
                )
                nc.tensor.matmul(
                    ps,
                    lhsT=sel_t[sa : sa + 1, 1, :],
                    rhs=recip_q[sa : sa + 1, 1, ncc, :],
                    start=False,
                    stop=True,
                    tile_position=(sa, 0),
                )
                nc.vector.tensor_mul(oT_u[:, a, nsl], oT_u[:, a, nsl], ps)

            # ---- output projection + residual + LayerNorm ----------------
            def out_tile(nt):
                ps = ps_mm.tile([128, 512], f32, tag="mm")
                for a in range(KC):
                    nc.tensor.matmul(
                        ps,
                        lhsT=oT_u[:, a, nt * 128 : (nt + 1) * 128],
                        rhs=wo[:, a, :],
                        start=(a == 0),
                        stop=(a == KC - 1),
                    )
                x_t = ypool.tile([128, D], f32, tag="x")
                nc.vector.tensor_add(x_t, ps, qres_sb[:, nt, :])
                stats = small.tile([128, 6], f32, tag="stats")
                nc.vector.bn_stats(out=stats, in_=x_t)
                mv = small.tile([128, 2], f32, tag="mv")
                nc.vector.bn_aggr(out=mv, in_=stats)
                # rstd = exp(-0.5 * ln(var + eps)); same ACT table set as
                # the softmax exp, so no table reloads anywhere
                lnv = small.tile([128, 1], f32, tag="lnv")
                nc.scalar.activation(lnv, mv[:, 1:2], Ln, bias=eps_t)
                rstd = small.tile([128, 1], f32, tag="rstd")
                nc.scalar.activation(rstd, lnv, Exp, scale=-0.5)
                nc.vector.tensor_scalar(
                    out=x_t, in0=x_t, scalar1=mv[:, 0:1], scalar2=rstd, op0=sub, op1=mult
                )
                if affine:
                    y_t = ypool.tile([128, D], f32, tag="y")
                    nc.gpsimd.tensor_mul(y_t, x_t, gamma_b)
                    nc.vector.tensor_add(y_t, y_t, beta_b)
                    nc.sync.dma_start(out=out_r[:, nt, :], in_=y_t)
                else:
                    nc.sync.dma_start(out=out_r[:, nt, :], in_=x_t)

            # ---- emission schedule ---------------------------------------
            q_proj(0, 0)
            k_proj(0, 0)

            def F(fn, *args):
                return lambda: fn(*args)

            f0 = {
                (0, 1): [F(v_proj, 0), F(v_proj, 1)], (0, 2): [F(v_proj, 2), F(v_proj, 3)],
                (0, 3): [F(k_proj, 0, 1)], (0, 5): [F(v_proj, 4), F(v_proj, 5)],
                (0, 6): [F(v_proj, 6), F(k_proj, 0, 2)], (0, 7): [F(v_proj, 7), F(v_proj, 8)],
                (0, 8): [F(v_proj, 9), F(k_proj, 0, 3)], (0, 9): [F(v_proj, 10), F(v_proj, 11)],
                (0, 10): [F(v_proj, 12), F(k_proj, 1, 0)], (0, 11): [F(v_proj, 13)],
                (0, 12): [F(v_proj, 14), F(q_proj, 1, 0)], (0, 13): [F(v_proj, 15)],
                (0, 14): [F(k_proj, 1, 1)], (0, 15): [F(k_proj, 1, 2)],
                (1, 0): [F(k_proj, 1, 3)], (1, 2): [F(k_proj, 2, 0)], (1, 5): [F(k_proj, 2, 1)],
                (1, 8): [F(k_proj, 2, 2)], (1, 11): [F(k_proj, 2, 3)], (1, 13): [F(q_proj, 2, 0)],
                (2, 2): [F(k_proj, 3, 0)], (2, 5): [F(k_proj, 3, 1)], (2, 8): [F(k_proj, 3, 2)],
                (2, 11): [F(k_proj, 3, 3)], (2, 13): [F(q_proj, 3, 0)],
                (3, 1): [F(q_proj, 0, 1)],
            }
            chunk(0, f0)

            # chunk 1: the boundary cluster (recips, norms, out-proj of the
            # first chunk, remaining Q projections) rides the exp-paced slack
            f1 = {
                (0, 1): [F(q_proj, 1, 1)],
                (0, 3): [F(recip, 0)],
                (0, 5): [F(norm, 0, 0)], (0, 7): [F(norm, 1, 0)],
                (0, 9): [F(norm, 2, 0)], (0, 11): [F(norm, 3, 0)],
                (1, 1): [F(out_tile, 0)], (1, 5): [F(q_proj, 2, 1)],
                (2, 1): [F(out_tile, 1)], (2, 3): [F(recip, 1, 0, 34)],
                (2, 5): [F(norm, 0, 1)], (2, 7): [F(norm, 1, 1)],
                (2, 9): [F(q_proj, 3, 1)],
                (3, 1): [F(out_tile, 2)], (3, 5): [F(out_tile, 3)],
            }
            chunk(1, f1)

            recip(1, 64, 98)          # pairs 2-3 of chunk 1
            # keep the PE warm across the recip/norm wait so the final
            # out-projection matmuls run at full clock
            for _ in range(6):
                psw = ps_mm.tile([128, 512], f32, tag="mm")
                for _ in range(2):
                    nc.tensor.matmul(
                        psw, lhsT=warm_mm[:, 0:128], rhs=warm_mm,
                        start=True, stop=True,
                    )
                nc.scalar.copy(warm[0:1, 1:2], psw[0:1, 0:1])
            norm(2, 1)
            norm(3, 1)
            # staged tail: two PSUM waves of out-proj matmuls, pipelined
            # residual adds + stats, ONE batched ln/exp for all four rstds,
            # then in-place normalizes + output DMAs
            xts, mvs = {}, {}
            for wave in (range(4, 6), range(6, 8)):
                pss = {}
                for nt in wave:
                    ps = ps_mm.tile([128, 512], f32, tag="mm")
                    for a in range(KC):
                        nc.tensor.matmul(
                            ps,
                            lhsT=oT_u[:, a, nt * 128 : (nt + 1) * 128],
                            rhs=wo[:, a, :],
                            start=(a == 0),
                            stop=(a == KC - 1),
                        )
                    pss[nt] = ps
                for nt in wave:
                    x_t = ypool.tile([128, D], f32, tag="x")
                    nc.vector.tensor_add(x_t, pss[nt], qres_sb[:, nt, :])
                    xts[nt] = x_t
            for nt in range(4, 8):
                stats = small.tile([128, 6], f32, tag="stats")
                nc.vector.bn_stats(out=stats, in_=xts[nt])
                mv = small.tile([128, 2], f32, tag="mv")
                nc.vector.bn_aggr(out=mv, in_=stats)
                mvs[nt] = mv
            var4 = small.tile([128, 4], f32, tag="var4")
            for nt in range(4, 8):
                nc.vector.tensor_copy(var4[:, nt - 4 : nt - 3], mvs[nt][:, 1:2])
            lnv4 = small.tile([128, 4], f32, tag="lnv4")
            nc.scalar.activation(lnv4, var4, Ln, bias=eps_t)
            rstd4 = small.tile([128, 4], f32, tag="rstd4")
            nc.scalar.activation(rstd4, lnv4, Exp, scale=-0.5)
            for nt in range(4, 8):
                nc.vector.tensor_scalar(
                    out=xts[nt], in0=xts[nt], scalar1=mvs[nt][:, 0:1],
                    scalar2=rstd4[:, nt - 4 : nt - 3], op0=sub, op1=mult,
                )
                if affine:
                    y_t = ypool.tile([128, D], f32, tag="y")
                    nc.gpsimd.tensor_mul(y_t, xts[nt], gamma_b)
                    nc.vector.tensor_add(y_t, y_t, beta_b)
                    nc.sync.dma_start(out=out_r[:, nt, :], in_=y_t)
                else:
                    nc.sync.dma_start(out=out_r[:, nt, :], in_=xts[nt])

    nc.compile()

    if os.environ.get("K_DUMP_TABLES"):
        import concourse.mybir as mybir2
        n_loads = sum(
            isinstance(i, mybir2.InstLoadActFuncSet)
            for b_ in nc.m.functions[0].blocks
            for i in b_.instructions
        )
        print(f"[build] InstLoadActFuncSet count: {n_loads}", file=sys.stderr)
    return nc


def kernel(**inputs):
    from concourse.bass_utils import run_bass_kernel_spmd

    query = np.asarray(inputs["query"], dtype=np.float32)
    key = np.asarray(inputs["key"], dtype=np.float32)
    value = np.asarray(inputs["value"], dtype=np.float32)
    mask = np.asarray(inputs["mask"])
    WQ = np.asarray(inputs["WQ"], dtype=np.float32)
    WK = np.asarray(inputs["WK"], dtype=np.float32)
    WV = np.asarray(inputs["WV"], dtype=np.float32)
    WO = np.asarray(inputs["WO"], dtype=np.float32)
    bO = np.asarray(inputs["bO"], dtype=np.float32)
    gamma = np.asarray(inputs["gamma"], dtype=np.float32)
    beta = np.asarray(inputs["beta"], dtype=np.float32)

    affine = not (np.all(gamma == 1.0) and np.all(beta == 0.0))
    ckey = ("nc", affine)
    if ckey not in _CACHE:
        _CACHE[ckey] = _build(affine)
    nc = _CACHE[ckey]

    scale = np.float32(1.0 / np.sqrt(HD))
    wqT = np.ascontiguousarray(WQ.T * scale).astype(BF16)
    wkT = np.ascontiguousarray(WK.T).astype(BF16)
    wvT = np.ascontiguousarray(WV.T).astype(BF16)
    woT = np.ascontiguousarray(WO.T).astype(BF16)
    mask_bin = (mask != 0)

    in_maps = []
    for c in range(NCORES):
        b, n0 = c // 2, (c % 2) * NS
        im = {
            "xqT": np.ascontiguousarray(query[b, n0 : n0 + NS, :].T).astype(BF16),
            "xkT": np.ascontiguousarray(key[b].T).astype(BF16),
            "xvT": np.ascontiguousarray(value[b].T).astype(BF16),
            "maskT": np.ascontiguousarray(mask_bin[b, n0 : n0 + NS, :].T).astype(BF16),
            "qres": np.ascontiguousarray(query[b, n0 : n0 + NS, :] + bO[None, :]).astype(BF16),
            "wqT": wqT, "wkT": wkT, "wvT": wvT, "woT": woT,
        }
        if affine:
            im["gamma"] = gamma.reshape(1, D)
            im["beta"] = beta.reshape(1, D)
        in_maps.append(im)

    trace = bool(int(os.environ.get("BASS_KERNEL_TRACE", "0")))
    res = run_bass_kernel_spmd(nc, in_maps, core_ids=list(range(NCORES)), trace=trace)
    _CACHE["last_results"] = res

    out = np.empty((B, N, D), dtype=np.float32)
    for c in range(NCORES):
        b, n0 = c // 2, (c % 2) * NS
        out[b, n0 : n0 + NS, :] = res.results[c]["out"]
    return out


# revision 40
# speedup vs baseline: 1.0707x; 1.0707x over previous
"""MultiHeadGraphAttention TRN2 kernel.

Data-parallel over (batch, query-half): core c handles batch c//2, query rows
(c%2)*1024 .. +1024.  Attention rows are independent, so there are no
collectives.  All matmuls run in bf16 (fp32 PSUM accumulation); softmax and
LayerNorm run in fp32.

Engine budget per core (the design target is max-engine, not sum):
  ScalarE: 128 exp ops of [128,1024] (the softmax) ~= 142us  <- pacer
  VectorE: mask multiplies (bf16 2x mode), PSUM evacuations, LayerNorm
  TensorE: projections + scores + AV + out-proj, kept dense (HAM-warm)

Key structural points:
  - scores are computed TRANSPOSED (S^T[m,n]) so softmax needs no on-chip
    transposes; the denominator comes free from a ones-column appended to V.
  - ScalarE uses ONLY {Exp, Ln} which live in one activation-table set
    (natural_log_exp_and_others): rsqrt(v) = exp(-0.5*ln(v)) and
    1/den = exp(-ln(den)).  No table switches anywhere.
  - per-head denominators are collected at partitions {32a, 32a+1} of a
    [98, ...] tile, reciprocated in 2 batched ScalarE ops per chunk, and
    broadcast to 128 partitions with a tiny contraction-2 PE matmul.
  - DMAs are emitted in consumption order (weights first); attention on
    head 0 starts as soon as Q(t=0)/K(t=0) land, with the remaining
    projections interleaved as fillers so the PE never idles.
"""

import os
import sys

import numpy as np

try:
    import concourse  # noqa: F401
except ImportError:  # harness runs from a bare dir; the repo is a fixed path
    sys.path.insert(0, "/opt/trn_rl_repo")

import ml_dtypes

B, N, M, D, H, HD = 4, 2048, 2048, 512, 8, 64
NS = 1024          # query rows per core
NCORES = 8
LN_EPS = 1e-5
BF16 = ml_dtypes.bfloat16

_CACHE = {}


def _build(affine: bool):
    import concourse.bass as bass  # noqa: F401
    import concourse.tile as tile
    from concourse import bacc, mybir

    f32 = mybir.dt.float32
    bf16 = mybir.dt.bfloat16
    Exp = mybir.ActivationFunctionType.Exp
    Ln = mybir.ActivationFunctionType.Ln

    # The act-table insertion pass picks the FIRST set containing each
    # function, which alternates exp_and_others / natural_log and inserts a
    # ~1.3us ACT_TABLE_LOAD per switch (23 of them).  Steer it to the one
    # set that contains BOTH exp and ln by stripping those functions from
    # the other sets in the cached table dict.  Keys/order are unchanged,
    # so act_func_set_id indices stay valid; the real on-device tables are
    # untouched (worst case is a redundant load, never wrong tables).
    from concourse.hw_specs import get_activation_tables
    sub = mybir.AluOpType.subtract
    mult = mybir.AluOpType.mult

    nc = bacc.Bacc(None, target_bir_lowering=False, debug=False)

    tables = get_activation_tables(nc.m.arch)
    combined = "natural_log_exp_and_others"
    if combined in tables:
        for name, funcs in tables.items():
            if name != combined:
                funcs.discard(Exp)
                funcs.discard(Ln)

    xqT_d = nc.dram_tensor("xqT", [D, NS], bf16, kind="ExternalInput")
    xkT_d = nc.dram_tensor("xkT", [D, M], bf16, kind="ExternalInput")
    xvT_d = nc.dram_tensor("xvT", [D, M], bf16, kind="ExternalInput")
    maskT_d = nc.dram_tensor("maskT", [M, NS], bf16, kind="ExternalInput")
    qres_d = nc.dram_tensor("qres", [NS, D], bf16, kind="ExternalInput")
    wqT_d = nc.dram_tensor("wqT", [D, D], bf16, kind="ExternalInput")
    wkT_d = nc.dram_tensor("wkT", [D, D], bf16, kind="ExternalInput")
    wvT_d = nc.dram_tensor("wvT", [D, D], bf16, kind="ExternalInput")
    woT_d = nc.dram_tensor("woT", [D, D], bf16, kind="ExternalInput")
    if affine:
        gamma_d = nc.dram_tensor("gamma", [1, D], f32, kind="ExternalInput")
        beta_d = nc.dram_tensor("beta", [1, D], f32, kind="ExternalInput")
    out_d = nc.dram_tensor("out", [NS, D], f32, kind="ExternalOutput")

    KC = D // 128      # 4 contraction chunks of 128
    NCH = NS // 512    # 2 query-column chunks
    MT = M // 128      # 16 key-position tiles
    MCH = M // 512     # 4 key chunks of 512

    xqT_r = xqT_d[:].rearrange("(c p) n -> p c n", p=128)
    xkT_r = xkT_d[:].rearrange("(c p) n -> p c n", p=128)
    xvT_r = xvT_d[:].rearrange("(c p) n -> p c n", p=128)
    maskT_r = maskT_d[:].rearrange("(j p) n -> p j n", p=128)
    qres_r = qres_d[:].rearrange("(t p) d -> p t d", p=128)
    out_r = out_d[:].rearrange("(t p) d -> p t d", p=128)

    with tile.TileContext(nc) as tc:
        with (
            tc.tile_pool(name="big", bufs=1) as big,
            tc.tile_pool(name="wpool", bufs=1) as wpool,
            tc.tile_pool(name="ppool", bufs=5) as ppool,
            tc.tile_pool(name="ypool", bufs=4) as ypool,
            tc.tile_pool(name="small", bufs=4) as small,
            tc.tile_pool(name="ps_s", bufs=2, space="PSUM") as ps_s,
            tc.tile_pool(name="ps_o", bufs=2, space="PSUM") as ps_o,
            tc.tile_pool(name="ps_mm", bufs=2, space="PSUM") as ps_mm,
        ):
            # ---- resident SBUF tensors -----------------------------------
            xqT = big.tile([128, KC, NS], bf16, tag="xqT")
            xkT = big.tile([128, KC, M], bf16, tag="xkT")
            xvT = big.tile([128, KC, M], bf16, tag="xvT")
            maskT = big.tile([128, MT, NS], bf16, tag="maskT")
            qT = big.tile([128, KC, NS], bf16, tag="qT")
            kT = big.tile([128, KC, M], bf16, tag="kT")
            vS = big.tile([128, MT, H * (HD + 1)], bf16, tag="vS")
            oT_u = big.tile([128, KC, NS], bf16, tag="oT_u")
            # denominators: head pair (2a, 2a+1) lives on partition 32a
            # (parity = second dim) so every engine AP base is 32-aligned
            den_q = big.tile([98, 2, NCH, 512], f32, tag="den_q")
            recip_q = big.tile([98, 2, NCH, 512], bf16, tag="recip_q")
            wq = wpool.tile([128, KC, D], bf16, tag="wq")
            wk = wpool.tile([128, KC, D], bf16, tag="wk")
            wv = wpool.tile([128, KC, D], bf16, tag="wv")
            wo = wpool.tile([128, KC, D], bf16, tag="wo")
            sel_t = wpool.tile([98, 2, 128], bf16, tag="sel_t")
            eps_t = wpool.tile([128, 1], f32, tag="eps")
            warm = wpool.tile([1, 2], f32, tag="warm")
            warm_mm = wpool.tile([128, 512], bf16, tag="warm_mm")
            qres_sb = big.tile([128, 8, 512], bf16, tag="qres_sb")
            if affine:
                gamma_1 = wpool.tile([1, D], f32, tag="gamma_1")
                beta_1 = wpool.tile([1, D], f32, tag="beta_1")
                gamma_b = wpool.tile([128, D], f32, tag="gamma_b")
                beta_b = wpool.tile([128, D], f32, tag="beta_b")

            # constants + ACT table warm-up (loads the ln/exp table set at
            # t~0, overlapping the input-DMA ramp)
            nc.vector.memset(eps_t, LN_EPS)
            nc.scalar.activation(warm[0:1, 0:1], eps_t[0:1, 0:1], Exp)
            nc.scalar.activation(warm[0:1, 1:2], eps_t[0:1, 0:1], Ln)
            # PE warm-up: ~4.5us of dummy matmuls before the first DMAs land
            # so HAM reaches K=8/8 by the time real projections start
            nc.vector.memset(warm_mm, 0.0)
            psw = ps_mm.tile([128, 512], f32, tag="mm")
            for i in range(24):
                nc.tensor.matmul(
                    psw, lhsT=warm_mm[:, 0:128], rhs=warm_mm,
                    start=(i == 0), stop=(i == 23),
                )
            # one reader keeps the whole accumulation group from DCE
            nc.scalar.copy(warm[0:1, 0:1], psw[0:1, 0:1])
            nc.vector.memset(sel_t, 0.0)
            for a in range(4):
                nc.vector.memset(sel_t[32 * a : 32 * a + 1, 0, 0:64], 1.0)
                nc.vector.memset(sel_t[32 * a : 32 * a + 1, 1, 64:128], 1.0)
            # ones column per head in augmented V -> softmax denominator
            # lands as row 64 of the AV matmul output
            nc.vector.memset(
                vS[:].rearrange("p j (h x) -> p j h x", x=HD + 1)[:, :, :, HD : HD + 1],
                1.0,
            )

            # ---- input DMAs in consumption order -------------------------
            def dma_mask(j):
                nc.sync.dma_start(out=maskT[:, j, :], in_=maskT_r[:, j, :])

            wq_r = wqT_d[:].rearrange("(c p) o -> p c o", p=128)
            wk_r = wkT_d[:].rearrange("(c p) o -> p c o", p=128)
            nc.sync.dma_start(out=wq[:, :, 0:128], in_=wq_r[:, :, 0:128])
            nc.sync.dma_start(out=xqT[:, :, 0:512], in_=xqT_r[:, :, 0:512])
            nc.sync.dma_start(out=wk[:, :, 0:128], in_=wk_r[:, :, 0:128])
            nc.sync.dma_start(out=xkT[:, :, 0:512], in_=xkT_r[:, :, 0:512])
            nc.sync.dma_start(out=wq[:, :, 128:512], in_=wq_r[:, :, 128:512])
            nc.sync.dma_start(out=wk[:, :, 128:512], in_=wk_r[:, :, 128:512])
            nc.sync.dma_start(out=wv, in_=wvT_d[:].rearrange("(c p) o -> p c o", p=128))
            nc.sync.dma_start(out=xvT[:, :, 0:512], in_=xvT_r[:, :, 0:512])
            nc.sync.dma_start(out=xkT[:, :, 512:1024], in_=xkT_r[:, :, 512:1024])
            dma_mask(0)
            dma_mask(1)
            nc.sync.dma_start(out=xvT[:, :, 512:1024], in_=xvT_r[:, :, 512:1024])
            nc.sync.dma_start(out=xkT[:, :, 1024:1536], in_=xkT_r[:, :, 1024:1536])
            dma_mask(2)
            dma_mask(3)
            nc.sync.dma_start(out=xkT[:, :, 1536:2048], in_=xkT_r[:, :, 1536:2048])
            nc.sync.dma_start(out=xvT[:, :, 1024:1536], in_=xvT_r[:, :, 1024:1536])
            dma_mask(4)
            dma_mask(5)
            nc.sync.dma_start(out=xvT[:, :, 1536:2048], in_=xvT_r[:, :, 1536:2048])
            for j in range(6, 10):
                dma_mask(j)
            nc.sync.dma_start(out=wo, in_=woT_d[:].rearrange("(c p) o -> p c o", p=128))
            nc.sync.dma_start(out=qres_sb, in_=qres_r)
            for j in range(10, MT):
                dma_mask(j)
            nc.sync.dma_start(out=xqT[:, :, 512:1024], in_=xqT_r[:, :, 512:1024])
            if affine:
                nc.sync.dma_start(out=gamma_1, in_=gamma_d[:])
                nc.sync.dma_start(out=beta_1, in_=beta_d[:])
                nc.gpsimd.partition_broadcast(gamma_b, gamma_1, channels=128)
                nc.gpsimd.partition_broadcast(beta_b, beta_1, channels=128)

            # ---- projection emitters (PSUM from ps_mm, evac on VectorE) --
            def q_proj(t, ncc):
                ps = ps_mm.tile([128, 512], f32, tag="mm")
                for kc in range(KC):
                    nc.tensor.matmul(
                        ps,
                        lhsT=wq[:, kc, t * 128 : (t + 1) * 128],
                        rhs=xqT[:, kc, ncc * 512 : (ncc + 1) * 512],
                        start=(kc == 0),
                        stop=(kc == KC - 1),
                    )
                nc.vector.tensor_copy(qT[:, t, ncc * 512 : (ncc + 1) * 512], ps)

            def k_proj(t, mc):
                ps = ps_mm.tile([128, 512], f32, tag="mm")
                for kc in range(KC):
                    nc.tensor.matmul(
                        ps,
                        lhsT=wk[:, kc, t * 128 : (t + 1) * 128],
                        rhs=xkT[:, kc, mc * 512 : (mc + 1) * 512],
                        start=(kc == 0),
                        stop=(kc == KC - 1),
                    )
                nc.vector.tensor_copy(kT[:, t, mc * 512 : (mc + 1) * 512], ps)

            def v_proj(j):
                ps = ps_mm.tile([128, 512], f32, tag="mm")
                for kc in range(KC):
                    nc.tensor.matmul(
                        ps,
                        lhsT=xvT[:, kc, j * 128 : (j + 1) * 128],
                        rhs=wv[:, kc, :],
                        start=(kc == 0),
                        stop=(kc == KC - 1),
                    )
                nc.vector.tensor_copy(
                    vS[:, j, :].rearrange("p (h x) -> p h x", x=HD + 1)[:, :, 0:HD],
                    ps[:].rearrange("p (h x) -> p h x", x=HD),
                )

            # ---- attention: one (head-pair, n-chunk) ---------------------
            # Heads 2p (PE rows 0-63) and 2p+1 (rows 64-127) are processed
            # together: their two K=64 score matmuls land in adjacent PE
            # queue slots on disjoint row groups, so the array runs them
            # CONCURRENTLY (row tiling) -- halving score PE time.
            def chunk(ncc, fillers):
                """All 4 head-pairs of one n-chunk, as a flat (pair, j) loop
                with a cross-pair AV pipeline: the next pair's score matmuls
                issue before the previous pair's last AVs + evacuation, so
                the PE stream never head-blocks at a pair boundary.
                fillers: {(p, j): [thunks]} emitted between scores and exp."""
                nsl = slice(ncc * 512, (ncc + 1) * 512)
                pend_av = []
                pend_evac = []

                def flush_av():
                    for pt_, j_, pa_, pb_, p_ in pend_av:
                        base = (j_ % 2) * 1024
                        for po_t, h, off in ((pa_, 2 * p_, 0), (pb_, 2 * p_ + 1, 512)):
                            nc.tensor.matmul(
                                po_t,
                                lhsT=vS[:, j_, h * (HD + 1) : (h + 1) * (HD + 1)],
                                rhs=pt_[:, base + off : base + off + 512],
                                start=(j_ == 0),
                                stop=(j_ == MT - 1),
                            )
                    pend_av.clear()

                def flush_evac():
                    for pa_, pb_, p_ in pend_evac:
                        nc.vector.tensor_copy(
                            den_q[32 * p_ : 32 * p_ + 1, 0, ncc, :], pa_[64:65, :]
                        )
                        nc.vector.tensor_copy(
                            den_q[32 * p_ : 32 * p_ + 1, 1, ncc, :], pb_[64:65, :]
                        )
                        nc.vector.tensor_copy(oT_u[0:64, p_, nsl], pa_[0:64, :])
                        nc.vector.tensor_copy(oT_u[64:128, p_, nsl], pb_[0:64, :])
                    pend_evac.clear()

                for p in range(4):
                    po_a = ps_o.tile([HD + 1, 512], f32, tag="po")
                    po_b = ps_o.tile([HD + 1, 512], f32, tag="po")
                    pt = None
                    for j in range(MT):
                        if j % 2 == 0:
                            pt = ppool.tile([128, 4 * 512], bf16, tag="pt")
                        psP = ps_s.tile([128, 1024], f32, tag="s")
                        nc.tensor.matmul(
                            psP[:, 0:512],
                            lhsT=kT[0:64, p, j * 128 : (j + 1) * 128],
                            rhs=qT[0:64, p, nsl],
                            start=True,
                            stop=True,
                        )
                        nc.tensor.matmul(
                            psP[:, 512:1024],
                            lhsT=kT[64:128, p, j * 128 : (j + 1) * 128],
                            rhs=qT[64:128, p, nsl],
                            start=True,
                            stop=True,
                        )
                        if j % 2 == 0:
                            # flush the previous j-pair's AVs one step earlier
                            # so pt tiles recycle sooner
                            flush_av()
                            flush_evac()
                        for f in fillers.get((p, j), ()):
                            f()
                        nc.scalar.activation(
                            pt[:, (j % 2) * 1024 : (j % 2 + 1) * 1024], psP, Exp
                        )
                        if j % 2 == 1:
                            ptv = pt[:].rearrange("p (a b n) -> p a b n", a=2, b=2)
                            nc.vector.tensor_mul(
                                ptv,
                                ptv,
                                maskT[:, j - 1 : j + 1, None, nsl].broadcast_to(
                                    [128, 2, 2, 512]
                                ),
                            )
                            flush_av()
                            flush_evac()
                            pend_av.append((pt, j - 1, po_a, po_b, p))
                            pend_av.append((pt, j, po_a, po_b, p))
                    pend_evac.append((po_a, po_b, p))
                flush_av()
                flush_evac()

            # ---- denominator reciprocal + normalize ----------------------
            def recip(ncc, p0=0, p1=98):
                # in-place ln, then recip = exp(-ln(den)); one ACT table set
                nc.scalar.activation(
                    den_q[p0:p1, :, ncc, :], den_q[p0:p1, :, ncc, :], Ln
                )
                nc.scalar.activation(
                    recip_q[p0:p1, :, ncc, :], den_q[p0:p1, :, ncc, :], Exp, scale=-1.0
                )

            def norm(a, ncc):
                nsl = slice(ncc * 512, (ncc + 1) * 512)
                sa = 32 * a
                ps = ps_mm.tile([128, 512], f32, tag="mm")
                # broadcast the pair's reciprocals to 128 partitions with two
                # chained K=1 matmuls: rows 0-63 <- head 2a, rows 64-127
                # (accumulated) <- head 2a+1
                nc.tensor.matmul(
                    ps,
                    lhsT=sel_t[sa : sa + 1, 0, :],
                    rhs=recip_q[sa : sa + 1, 0, ncc, :],
                    start=True,
                    stop=False,
                    tile_position=(sa, 0),
                )
                nc.tensor.matmul(
                    ps,
                    lhsT=sel_t[sa : sa + 1, 1, :],
                    rhs=recip_q[sa : sa + 1, 1, ncc, :],
                    start=False,
                    stop=True,
                    tile_position=(sa, 0),
                )
                nc.vector.tensor_mul(oT_u[:, a, nsl], oT_u[:, a, nsl], ps)

            # ---- output projection + residual + LayerNorm ----------------
            def out_tile(nt):
                ps = ps_mm.tile([128, 512], f32, tag="mm")
                for a in range(KC):
                    nc.tensor.matmul(
                        ps,
                        lhsT=oT_u[:, a, nt * 128 : (nt + 1) * 128],
                        rhs=wo[:, a, :],
                        start=(a == 0),
                        stop=(a == KC - 1),
                    )
                x_t = ypool.tile([128, D], f32, tag="x")
                nc.vector.tensor_add(x_t, ps, qres_sb[:, nt, :])
                stats = small.tile([128, 6], f32, tag="stats")
                nc.vector.bn_stats(out=stats, in_=x_t)
                mv = small.tile([128, 2], f32, tag="mv")
                nc.vector.bn_aggr(out=mv, in_=stats)
                # rstd = exp(-0.5 * ln(var + eps)); same ACT table set as
                # the softmax exp, so no table reloads anywhere
                lnv = small.tile([128, 1], f32, tag="lnv")
                nc.scalar.activation(lnv, mv[:, 1:2], Ln, bias=eps_t)
                rstd = small.tile([128, 1], f32, tag="rstd")
                nc.scalar.activation(rstd, lnv, Exp, scale=-0.5)
                nc.vector.tensor_scalar(
                    out=x_t, in0=x_t, scalar1=mv[:, 0:1], scalar2=rstd, op0=sub, op1=mult
                )
                if affine:
                    y_t = ypool.tile([128, D], f32, tag="y")
                    nc.gpsimd.tensor_mul(y_t, x_t, gamma_b)
                    nc.vector.tensor_add(y_t, y_t, beta_b)
                    nc.sync.dma_start(out=out_r[:, nt, :], in_=y_t)
                else:
                    nc.sync.dma_start(out=out_r[:, nt, :], in_=x_t)

            # ---- emission schedule ---------------------------------------
            q_proj(0, 0)
            k_proj(0, 0)

            def F(fn, *args):
                return lambda: fn(*args)

            f0 = {
                (0, 1): [F(v_proj, 0), F(v_proj, 1)], (0, 2): [F(v_proj, 2), F(v_proj, 3)],
                (0, 3): [F(k_proj, 0, 1)], (0, 5): [F(v_proj, 4), F(v_proj, 5)],
                (0, 6): [F(v_proj, 6), F(k_proj, 0, 2)], (0, 7): [F(v_proj, 7), F(v_proj, 8)],
                (0, 8): [F(v_proj, 9), F(k_proj, 0, 3)], (0, 9): [F(v_proj, 10), F(v_proj, 11)],
                (0, 10): [F(v_proj, 12), F(k_proj, 1, 0)], (0, 11): [F(v_proj, 13)],
                (0, 12): [F(v_proj, 14), F(q_proj, 1, 0)], (0, 13): [F(v_proj, 15)],
                (0, 14): [F(k_proj, 1, 1)], (0, 15): [F(k_proj, 1, 2)],
                (1, 0): [F(k_proj, 1, 3)], (1, 2): [F(k_proj, 2, 0)], (1, 5): [F(k_proj, 2, 1)],
                (1, 8): [F(k_proj, 2, 2)], (1, 11): [F(k_proj, 2, 3)], (1, 13): [F(q_proj, 2, 0)],
                (2, 2): [F(k_proj, 3, 0)], (2, 5): [F(k_proj, 3, 1)], (2, 8): [F(k_proj, 3, 2)],
                (2, 11): [F(k_proj, 3, 3)], (2, 13): [F(q_proj, 3, 0)],
                (3, 1): [F(q_proj, 0, 1)],
            }
            chunk(0, f0)

            # chunk 1: the boundary cluster (recips, norms, out-proj of the
            # first chunk, remaining Q projections) rides the exp-paced slack
            f1 = {
                (0, 1): [F(q_proj, 1, 1)],
                (0, 3): [F(recip, 0)],
                (0, 5): [F(norm, 0, 0)], (0, 7): [F(norm, 1, 0)],
                (0, 9): [F(norm, 2, 0)], (0, 11): [F(norm, 3, 0)],
                (1, 1): [F(out_tile, 0)], (1, 5): [F(q_proj, 2, 1)],
                (2, 1): [F(out_tile, 1)], (2, 3): [F(recip, 1, 0, 34)],
                (2, 5): [F(norm, 0, 1)], (2, 7): [F(norm, 1, 1)],
                (2, 9): [F(q_proj, 3, 1)],
                (3, 1): [F(out_tile, 2)], (3, 5): [F(out_tile, 3)],
            }
            chunk(1, f1)

            recip(1, 64, 98)          # pairs 2-3 of chunk 1
            # keep the PE warm across the recip/norm wait so the final
            # out-projection matmuls run at full clock
            psw2 = ps_mm.tile([128, 512], f32, tag="mm")
            for i in range(12):
                nc.tensor.matmul(
                    psw2, lhsT=warm_mm[:, 0:128], rhs=warm_mm,
                    start=(i == 0), stop=(i == 11),
                )
            nc.scalar.copy(warm[0:1, 1:2], psw2[0:1, 0:1])
            norm(2, 1)
            norm(3, 1)
            # staged tail: two PSUM waves of out-proj matmuls, pipelined
            # residual adds + stats, ONE batched ln/exp for all four rstds,
            # then in-place normalizes + output DMAs
            xts, mvs = {}, {}
            for wave in (range(4, 6), range(6, 8)):
                pss = {}
                for nt in wave:
                    ps = ps_mm.tile([128, 512], f32, tag="mm")
                    for a in range(KC):
                        nc.tensor.matmul(
                            ps,
                            lhsT=oT_u[:, a, nt * 128 : (nt + 1) * 128],
                            rhs=wo[:, a, :],
                            start=(a == 0),
                            stop=(a == KC - 1),
                        )
                    pss[nt] = ps
                for nt in wave:
                    x_t = ypool.tile([128, D], f32, tag="x")
                    nc.vector.tensor_add(x_t, pss[nt], qres_sb[:, nt, :])
                    xts[nt] = x_t
            for nt in range(4, 8):
                stats = small.tile([128, 6], f32, tag="stats")
                nc.vector.bn_stats(out=stats, in_=xts[nt])
                mv = small.tile([128, 2], f32, tag="mv")
                nc.vector.bn_aggr(out=mv, in_=stats)
                mvs[nt] = mv
            var4 = small.tile([128, 4], f32, tag="var4")
            for nt in range(4, 8):
                nc.vector.tensor_copy(var4[:, nt - 4 : nt - 3], mvs[nt][:, 1:2])
            lnv4 = small.tile([128, 4], f32, tag="lnv4")
            nc.scalar.activation(lnv4, var4, Ln, bias=eps_t)
            rstd4 = small.tile([128, 4], f32, tag="rstd4")
            nc.scalar.activation(rstd4, lnv4, Exp, scale=-0.5)
            for nt in range(4, 8):
                nc.vector.tensor_scalar(
                    out=xts[nt], in0=xts[nt], scalar1=mvs[nt][:, 0:1],
                    scalar2=rstd4[:, nt - 4 : nt - 3], op0=sub, op1=mult,
                )
                if affine:
                    y_t = ypool.tile([128, D], f32, tag="y")
                    nc.gpsimd.tensor_mul(y_t, xts[nt], gamma_b)
                    nc.vector.tensor_add(y_t, y_t, beta_b)
                    nc.sync.dma_start(out=out_r[:, nt, :], in_=y_t)
                else:
                    nc.sync.dma_start(out=out_r[:, nt, :], in_=xts[nt])

    nc.compile()

    if os.environ.get("K_DUMP_TABLES"):
        import concourse.mybir as mybir2
        n_loads = sum(
            isinstance(i, mybir2.InstLoadActFuncSet)
            for b_ in nc.m.functions[0].blocks
            for i in b_.instructions
        )
        print(f"[build] InstLoadActFuncSet count: {n_loads}", file=sys.stderr)
    return nc


def kernel(**inputs):
    from concourse.bass_utils import run_bass_kernel_spmd

    query = np.asarray(inputs["query"], dtype=np.float32)
    key = np.asarray(inputs["key"], dtype=np.float32)
    value = np.asarray(inputs["value"], dtype=np.float32)
    mask = np.asarray(inputs["mask"])
    WQ = np.asarray(inputs["WQ"], dtype=np.float32)
    WK = np.asarray(inputs["WK"], dtype=np.float32)
    WV = np.asarray(inputs["WV"], dtype=np.float32)
    WO = np.asarray(inputs["WO"], dtype=np.float32)
    bO = np.asarray(inputs["bO"], dtype=np.float32)
    gamma = np.asarray(inputs["gamma"], dtype=np.float32)
    beta = np.asarray(inputs["beta"], dtype=np.float32)

    affine = not (np.all(gamma == 1.0) and np.all(beta == 0.0))
    ckey = ("nc", affine)
    if ckey not in _CACHE:
        _CACHE[ckey] = _build(affine)
    nc = _CACHE[ckey]

    scale = np.float32(1.0 / np.sqrt(HD))
    wqT = np.ascontiguousarray(WQ.T * scale).astype(BF16)
    wkT = np.ascontiguousarray(WK.T).astype(BF16)
    wvT = np.ascontiguousarray(WV.T).astype(BF16)
    woT = np.ascontiguousarray(WO.T).astype(BF16)
    mask_bin = (mask != 0)

    in_maps = []
    for c in range(NCORES):
        b, n0 = c // 2, (c % 2) * NS
        im = {
            "xqT": np.ascontiguousarray(query[b, n0 : n0 + NS, :].T).astype(BF16),
            "xkT": np.ascontiguousarray(key[b].T).astype(BF16),
            "xvT": np.ascontiguousarray(value[b].T).astype(BF16),
            "maskT": np.ascontiguousarray(mask_bin[b, n0 : n0 + NS, :].T).astype(BF16),
            "qres": np.ascontiguousarray(query[b, n0 : n0 + NS, :] + bO[None, :]).astype(BF16),
            "wqT": wqT, "wkT": wkT, "wvT": wvT, "woT": woT,
        }
        if affine:
            im["gamma"] = gamma.reshape(1, D)
            im["beta"] = beta.reshape(1, D)
        in_maps.append(im)

    trace = bool(int(os.environ.get("BASS_KERNEL_TRACE", "0")))
    res = run_bass_kernel_spmd(nc, in_maps, core_ids=list(range(NCORES)), trace=trace)
    _CACHE["last_results"] = res

    out = np.empty((B, N, D), dtype=np.float32)
    for c in range(NCORES):
        b, n0 = c // 2, (c % 2) * NS
        out[b, n0 : n0 + NS, :] = res.results[c]["out"]
    return out
